# revision 8
# baseline (speedup 1.0000x reference)
"""AttentionBlock kernel for 8 TRN2 NeuronCores.

Data-parallel over batch: core b computes the full attention block for
batch element b (B=8, one per core). No collectives.

Per-core math (C=512, L=1024, 32 groups, 8 heads, ch=64):
  xn   = GroupNorm(x)                                  [C, L]
  q,k  = W_q xn + b_q, W_k xn + b_k (head-major rows)  [512, L] each
  vT   = xn^T W_v^T + b_v (computed directly as [s,c]) [L, 512]
  WT_h[s,t] = sum_c k_h[c,s] q_h[c,t]                  (per head)
  ET   = exp(WT/8)  (ACT, fused row-sum accum -> N[s])
  a_h[c,t] = sum_s (vT_h[s,c]/N_h[s]) ET_h[s,t]        (softmax over t)
  out  = W_p a + b_p + x

Schedule: attention is processed per head-pair (2j, 2j+1) x s-chunk,
with the AV matmuls delayed one chunk behind the QK/exp front and the
next pair's q/k projection matmuls interleaved one per chunk, so the
PE stays dense (HAM stays warm) while the ACT engine (exp+rowsum, the
per-core bottleneck) is never starved.
"""

import os
import sys

for _p in ("/opt/trn_rl_repo",):
    if os.path.isdir(_p) and _p not in sys.path:
        sys.path.insert(0, _p)

import numpy as np
import ml_dtypes

B, C, HS, WS = 8, 512, 32, 32
L = HS * WS           # 1024
NG = 32               # groups
GSZ = C // NG         # 16 channels per group
NH = 8                # heads
CPH = C // NH         # 64 channels per head
EPS = 1e-5
NCORES = 8

_CACHE = {}


def _head_perm():
    """perm[new] = old row of the 1536-row qkv weight: q all heads, then k, then v."""
    q, k, v = [], [], []
    for h in range(NH):
        base = h * 3 * CPH
        q.extend(range(base, base + CPH))
        k.extend(range(base + CPH, base + 2 * CPH))
        v.extend(range(base + 2 * CPH, base + 3 * CPH))
    return np.array(q + k + v, dtype=np.int64)


def _const_mats():
    # emat[j, p, g] = 1 if group(128j + p) == g   (group stats gather)
    emat = np.zeros((4, 128, NG), dtype=np.float32)
    for j in range(4):
        for p in range(128):
            emat[j, p, (128 * j + p) // GSZ] = 1.0
    # bmat[g, p]: out[p, j] = sum_g bmat[g,p] * R[g,j] -> picks g = 8j + p//16
    bmat = np.zeros((NG, 128), dtype=np.float32)
    for g in range(NG):
        for p in range(128):
            if g % 8 == p // GSZ:
                bmat[g, p] = 1.0
    # m8[g, j] = 1 if g//8 == j  (chunk mask)
    m8 = np.zeros((NG, 4), dtype=np.float32)
    for g in range(NG):
        m8[g, g // 8] = 1.0
    return emat, bmat, m8


def build_nc():
    import concourse.bacc as bacc
    import concourse.tile as tile
    from concourse import mybir

    f32 = mybir.dt.float32
    bf16 = mybir.dt.bfloat16
    AF = mybir.ActivationFunctionType
    OP = mybir.AluOpType

    nc = bacc.Bacc("TRN2", target_bir_lowering=False, debug=False,
                   num_devices=NCORES)

    x_d = nc.declare_dram_parameter("x", [C, L], f32, isOutput=False)
    wqkvT_d = nc.declare_dram_parameter("wqkvT", [C, 3 * C], bf16, isOutput=False)
    wprojT_d = nc.declare_dram_parameter("wprojT", [C, C], bf16, isOutput=False)
    bqk_d = nc.declare_dram_parameter("bqk", [8, 128], f32, isOutput=False)
    bv_d = nc.declare_dram_parameter("bv", [C], f32, isOutput=False)
    gnw_d = nc.declare_dram_parameter("gnw", [4, 128], f32, isOutput=False)
    gnb_d = nc.declare_dram_parameter("gnb", [4, 128], f32, isOutput=False)
    bproj_d = nc.declare_dram_parameter("bproj", [4, 128], f32, isOutput=False)
    emat_d = nc.declare_dram_parameter("emat", [4, 128, NG], f32, isOutput=False)
    bmat_d = nc.declare_dram_parameter("bmat", [NG, 128], f32, isOutput=False)
    m8_d = nc.declare_dram_parameter("m8", [NG, 4], f32, isOutput=False)
    out_d = nc.declare_dram_parameter("out", [C, L], f32, isOutput=True)

    import concourse.bass as bass
    from contextlib import ExitStack

    with tile.TileContext(nc) as tc, ExitStack() as ctx:
        tc.race_detector_enabled = False
        P = ctx.enter_context(tc.tile_pool(name="persist", bufs=1))
        work = ctx.enter_context(tc.tile_pool(name="work", bufs=4))
        etp = ctx.enter_context(tc.tile_pool(name="et", bufs=8))
        ostp = ctx.enter_context(tc.tile_pool(name="ost", bufs=4))
        psA = ctx.enter_context(tc.tile_pool(name="psA", bufs=2, space="PSUM"))
        psB = ctx.enter_context(tc.tile_pool(name="psB", bufs=2, space="PSUM"))

        # ---- persistent tiles
        x_sb = P.tile([128, 4, L], f32, tag="x", name="x")
        xn_sb = P.tile([128, 4, L], bf16, tag="xn", name="xn")
        wq_sb = [P.tile([128, 3 * C], bf16, tag=f"wqkvT{i}", name=f"wqkvT{i}")
                 for i in range(4)]
        wp_sb = [P.tile([128, C], bf16, tag=f"wprojT{i}", name=f"wprojT{i}")
                 for i in range(4)]
        qk_sb = [P.tile([128, L], bf16, tag=f"qk{j}", name=f"qk{j}") for j in range(8)]
        vt_sb = [P.tile([128, C], bf16, tag=f"vt{s}", name=f"vt{s}") for s in range(8)]
        a_sb = [P.tile([128, L], bf16, tag=f"a{j}", name=f"a{j}") for j in range(4)]
        bqk_sb = P.tile([128, 8], f32, tag="bqk", name="bqk")
        bv_sb = P.tile([128, C], f32, tag="bv", name="bv")
        gnw_sb = P.tile([128, 4], f32, tag="gnw", name="gnw")
        gnb_sb = P.tile([128, 4], f32, tag="gnb", name="gnb")
        bproj_sb = P.tile([128, 4], f32, tag="bproj", name="bproj")
        emat_sb = P.tile([128, 4, NG], f32, tag="emat", name="emat")
        bmat_sb = P.tile([NG, 128], f32, tag="bmat", name="bmat")
        m8_sb = P.tile([NG, 4], f32, tag="m8", name="m8")
        zeros_sb = P.tile([128, 1], f32, tag="zeros", name="zeros")
        eps_sb = P.tile([NG, 1], f32, tag="eps", name="eps")
        N_sb = [P.tile([128, 16], f32, tag=f"N{j}", name=f"N{j}") for j in range(4)]
        rN_sb = [P.tile([128, 16], f32, tag=f"rN{j}", name=f"rN{j}")
                 for j in range(4)]
        acc_sb = [P.tile([128, L], f32, tag=f"acc{o}", name=f"acc{o}")
                  for o in range(4)]

        # ---- input DMA (x first: groupnorm is on the critical path)
        for j in range(4):
            for sub in range(2):
                nc.sync.dma_start(
                    out=x_sb[:, j, 512 * sub:512 * (sub + 1)],
                    in_=x_d[128 * j:128 * (j + 1), 512 * sub:512 * (sub + 1)])
        for i in range(4):
            nc.sync.dma_start(out=wq_sb[i], in_=wqkvT_d[128 * i:128 * (i + 1), :])
        nc.sync.dma_start(out=bqk_sb, in_=bqk_d.ap().rearrange("a p -> p a"))
        bv_ap = bass.AP(tensor=bv_d, offset=0, ap=[[0, 128], [1, C]])
        nc.sync.dma_start(out=bv_sb, in_=bv_ap)
        nc.sync.dma_start(out=gnw_sb, in_=gnw_d.ap().rearrange("a p -> p a"))
        nc.sync.dma_start(out=gnb_sb, in_=gnb_d.ap().rearrange("a p -> p a"))
        nc.sync.dma_start(out=bproj_sb, in_=bproj_d.ap().rearrange("a p -> p a"))
        nc.sync.dma_start(out=emat_sb, in_=emat_d.ap().rearrange("a p g -> p a g"))
        nc.sync.dma_start(out=bmat_sb, in_=bmat_d[:, :])
        nc.sync.dma_start(out=m8_sb, in_=m8_d[:, :])
        for i in range(4):
            nc.sync.dma_start(out=wp_sb[i], in_=wprojT_d[128 * i:128 * (i + 1), :])
        nc.vector.memset(zeros_sb, 0.0)
        nc.vector.memset(eps_sb, EPS)
        warm = work.tile([NG, 1], f32, tag="warm", name="warm")
        nc.scalar.activation(out=warm, in_=eps_sb, func=AF.Ln, bias=eps_sb,
                             scale=1.0)

        # ---- phase 1: GroupNorm
        gstats = psA.tile([NG, 2], f32, tag="psA", name="psA_gs")
        for j in range(4):
            st = work.tile([128, 2, 6], f32, tag="bnst", name="bnst")
            for sub in range(2):
                nc.vector.bn_stats(out=st[:, sub, :],
                                   in_=x_sb[:, j, 512 * sub:512 * (sub + 1)])
            mv = work.tile([128, 2], f32, tag="bnmv", name="bnmv")
            nc.vector.bn_aggr(out=mv, in_=st)
            mm2 = work.tile([128, 2], f32, tag="mm2", name="mm2")  # [mean, mean^2+var]
            nc.vector.tensor_copy(out=mm2[:, 0:1], in_=mv[:, 0:1])
            nc.vector.tensor_mul(out=mm2[:, 1:2], in0=mv[:, 0:1], in1=mv[:, 0:1])
            nc.vector.tensor_add(out=mm2[:, 1:2], in0=mm2[:, 1:2], in1=mv[:, 1:2])
            nc.tensor.matmul(gstats, lhsT=emat_sb[:, j, :], rhs=mm2,
                             start=(j == 0), stop=(j == 3))
        gs = work.tile([NG, 2], f32, tag="gs", name="gs")
        nc.vector.tensor_scalar_mul(out=gs, in0=gstats, scalar1=1.0 / GSZ)
        gvar = work.tile([NG, 1], f32, tag="gvar", name="gvar")
        nc.vector.tensor_mul(out=gvar, in0=gs[:, 0:1], in1=gs[:, 0:1])
        nc.vector.tensor_sub(out=gvar, in0=gs[:, 1:2], in1=gvar)
        lnv = work.tile([NG, 1], f32, tag="lnv", name="lnv")
        nc.scalar.activation(out=lnv, in_=gvar, func=AF.Ln, bias=eps_sb, scale=1.0)
        rstd = work.tile([NG, 1], f32, tag="rstd", name="rstd")
        nc.scalar.activation(out=rstd, in_=lnv, func=AF.Exp, bias=zeros_sb[:NG],
                             scale=-0.5)
        R = work.tile([NG, 8], f32, tag="R", name="R")
        nc.vector.tensor_scalar_mul(out=R[:, 0:4], in0=m8_sb, scalar1=gs[:, 0:1])
        nc.vector.tensor_scalar_mul(out=R[:, 4:8], in0=m8_sb, scalar1=rstd)
        pc = psA.tile([128, 8], f32, tag="psA", name="psA_pc")
        nc.tensor.matmul(pc, lhsT=bmat_sb, rhs=R, start=True, stop=True)
        scale = work.tile([128, 4], f32, tag="scale", name="scale")
        shift = work.tile([128, 4], f32, tag="shift", name="shift")
        nc.vector.tensor_mul(out=scale, in0=gnw_sb, in1=pc[:, 4:8])
        nc.vector.tensor_mul(out=shift, in0=pc[:, 0:4], in1=scale)
        nc.vector.tensor_sub(out=shift, in0=gnb_sb, in1=shift)
        for j in range(4):
            nc.vector.tensor_scalar(out=xn_sb[:, j, :], in0=x_sb[:, j, :],
                                    scalar1=scale[:, j:j + 1],
                                    scalar2=shift[:, j:j + 1],
                                    op0=OP.mult, op1=OP.add)

        # helpers -----------------------------------------------------
        pools = [psA, psB]

        def vt_unit(s, pool):
            ps = pool.tile([128, C], f32, tag=pool.name, name="ps_vt")
            for i in range(4):
                nc.tensor.matmul(ps,
                                 lhsT=xn_sb[:, i, 128 * s:128 * (s + 1)],
                                 rhs=wq_sb[i][:, 2 * C:3 * C],
                                 start=(i == 0), stop=(i == 3))
            nc.vector.tensor_add(out=vt_sb[s], in0=ps, in1=bv_sb)

        def qk_half_unit(jj, n, pool):
            ps = pool.tile([128, 512], f32, tag=pool.name, name="ps_qkh")
            for i in range(4):
                nc.tensor.matmul(ps,
                                 lhsT=wq_sb[i][:, 128 * jj:128 * (jj + 1)],
                                 rhs=xn_sb[:, i, 512 * n:512 * (n + 1)],
                                 start=(i == 0), stop=(i == 3))
            nc.vector.tensor_scalar_add(out=qk_sb[jj][:, 512 * n:512 * (n + 1)],
                                        in0=ps, scalar1=bqk_sb[:, jj:jj + 1])

        # ---- phase 2 lead-in: q0/k0 and the first two vT chunks
        for u, (jj, n) in enumerate(((0, 0), (0, 1), (4, 0), (4, 1))):
            qk_half_unit(jj, n, pools[u % 2])
        vt_unit(0, psA)
        vt_unit(1, psB)
        for j in range(4):
            # residual gets b_proj folded in: x <- x + b_proj (off critical path)
            nc.vector.tensor_scalar_add(out=x_sb[:, j, :], in0=x_sb[:, j, :],
                                        scalar1=bproj_sb[:, j:j + 1])

        # ---- phase 3: attention; per pair (heads 2j / 2j+1), AV lags 2 chunks.
        # Remaining vT chunks and the next pair's q/k projections are
        # interleaved as transient psum units to keep the PE dense.
        et_tiles = {}
        vtn_tiles = {}

        def front(j, c):
            for hh, po in ((0, 0), (1, 64)):
                h = 2 * j + hh
                qp = psB.tile([128, L], f32, tag="psB", name="psB_qk")
                for n in range(2):
                    nc.tensor.matmul(qp[:, 512 * n:512 * (n + 1)],
                                     lhsT=qk_sb[4 + j][po:po + 64,
                                                       128 * c:128 * (c + 1)],
                                     rhs=qk_sb[j][po:po + 64,
                                                  512 * n:512 * (n + 1)],
                                     start=True, stop=True)
                et = etp.tile([128, L], bf16, tag="et", name="et")
                nc.scalar.activation(out=et, in_=qp, func=AF.Exp, bias=zeros_sb,
                                     scale=0.125,
                                     accum_out=N_sb[j][:, 2 * c + hh:2 * c + hh + 1])
                et_tiles[(h, c)] = et
            nc.vector.reciprocal(out=rN_sb[j][:, 2 * c:2 * c + 2],
                                 in_=N_sb[j][:, 2 * c:2 * c + 2])
            for hh in (0, 1):
                h = 2 * j + hh
                vtn = work.tile([128, 64], bf16, tag="vtn", name="vtn", bufs=8)
                nc.gpsimd.tensor_scalar_mul(
                    out=vtn, in0=vt_sb[c][:, CPH * h:CPH * (h + 1)],
                    scalar1=rN_sb[j][:, 2 * c + hh:2 * c + hh + 1])
                vtn_tiles[(h, c)] = vtn

        def back(j, c, av):
            for h, po in ((2 * j, 0), (2 * j + 1, 64)):
                vtn = vtn_tiles.pop((h, c))
                et = et_tiles.pop((h, c))
                for n in range(2):
                    nc.tensor.matmul(av[po:po + 64, 512 * n:512 * (n + 1)],
                                     lhsT=vtn, rhs=et[:, 512 * n:512 * (n + 1)],
                                     start=(c == 0), stop=(c == 7),
                                     skip_group_check=True)

        def proj_unit(i, o):
            # partial proj contribution of a_sb[i] to output row-block o
            ps = psA.tile([128, L], f32, tag="psA", name="ps_proj")
            for n in range(2):
                nc.tensor.matmul(ps[:, 512 * n:512 * (n + 1)],
                                 lhsT=wp_sb[i][:, 128 * o:128 * (o + 1)],
                                 rhs=a_sb[i][:, 512 * n:512 * (n + 1)],
                                 start=True, stop=True)
            if i == 0:
                # seed with residual (x already has b_proj folded in)
                nc.vector.tensor_add(out=acc_sb[o], in0=ps, in1=x_sb[:, o, :])
            else:
                nc.vector.tensor_add(out=acc_sb[o], in0=ps, in1=acc_sb[o])

        # insert schedule: pair -> chunk -> list of units
        ins_sched = {
            0: {0: [("vt", 2), ("qk", 1, 0)], 1: [("vt", 3), ("qk", 1, 1)],
                2: [("vt", 4), ("qk", 5, 0)], 3: [("vt", 5), ("qk", 5, 1)],
                4: [("vt", 6)], 5: [("vt", 7)]},
            1: {0: [("qk", 2, 0)], 1: [("qk", 2, 1)],
                2: [("qk", 6, 0)], 3: [("qk", 6, 1)],
                4: [("pr", 0, 0)], 5: [("pr", 0, 1)],
                6: [("pr", 0, 2)], 7: [("pr", 0, 3)]},
            2: {0: [("qk", 3, 0)], 1: [("qk", 3, 1)],
                2: [("qk", 7, 0)], 3: [("qk", 7, 1)],
                4: [("pr", 1, 0)], 5: [("pr", 1, 1)],
                6: [("pr", 1, 2)], 7: [("pr", 1, 3)]},
            3: {0: [("pr", 2, 0)], 1: [("pr", 2, 1)],
                2: [("pr", 2, 2)], 3: [("pr", 2, 3)]},
        }

        DELAY = 2
        for j in range(4):
            av = psA.tile([128, L], f32, tag="psA", name="psA_av")
            for c in range(8):
                front(j, c)
                for unit in ins_sched[j].get(c, ()):
                    if unit[0] == "vt":
                        vt_unit(unit[1], psA)
                    elif unit[0] == "pr":
                        proj_unit(unit[1], unit[2])
                    else:
                        qk_half_unit(unit[1], unit[2], psA)
                if c >= DELAY:
                    back(j, c - DELAY, av)
            for c in range(8 - DELAY, 8):
                back(j, c, av)
            nc.vector.tensor_copy(out=a_sb[j], in_=av)

        # ---- phase 4: last proj partial + store
        for o in range(4):
            ps = psA.tile([128, L], f32, tag="psA", name="psA_pr")
            for n in range(2):
                nc.tensor.matmul(ps[:, 512 * n:512 * (n + 1)],
                                 lhsT=wp_sb[3][:, 128 * o:128 * (o + 1)],
                                 rhs=a_sb[3][:, 512 * n:512 * (n + 1)],
                                 start=True, stop=True)
            ot = ostp.tile([128, L], f32, tag="ost", name="ost")
            nc.vector.tensor_add(out=ot, in0=ps, in1=acc_sb[o])
            nc.sync.dma_start(out=out_d[128 * o:128 * (o + 1), :], in_=ot)

    return nc


def prep_inputs(x, gn_w, gn_b, w_qkv, b_qkv, w_proj, b_proj):
    """Host-side prep: permute/transpose/cast; returns per-core in_maps."""
    x = np.asarray(x, dtype=np.float32)
    gn_w = np.asarray(gn_w, dtype=np.float32)
    gn_b = np.asarray(gn_b, dtype=np.float32)
    w_qkv = np.asarray(w_qkv, dtype=np.float32)
    b_qkv = np.asarray(b_qkv, dtype=np.float32)
    w_proj = np.asarray(w_proj, dtype=np.float32)
    b_proj = np.asarray(b_proj, dtype=np.float32)

    perm = _head_perm()
    wqkvT = np.ascontiguousarray(w_qkv[perm].T).astype(ml_dtypes.bfloat16)
    b_perm = b_qkv[perm]
    wprojT = np.ascontiguousarray(w_proj.T).astype(ml_dtypes.bfloat16)
    emat, bmat, m8 = _const_mats()

    shared = {
        "wqkvT": wqkvT,
        "wprojT": wprojT,
        "bqk": np.ascontiguousarray(b_perm[:1024].reshape(8, 128)),
        "bv": np.ascontiguousarray(b_perm[1024:]),
        "gnw": np.ascontiguousarray(gn_w.reshape(4, 128)),
        "gnb": np.ascontiguousarray(gn_b.reshape(4, 128)),
        "bproj": np.ascontiguousarray(b_proj.reshape(4, 128)),
        "emat": emat, "bmat": bmat, "m8": m8,
    }
    xf = x.reshape(B, C, L)
    in_maps = [dict(shared, x=np.ascontiguousarray(xf[b])) for b in range(B)]
    return in_maps


def kernel(x, gn_w, gn_b, w_qkv, b_qkv, w_proj, b_proj):
    from concourse.bass_utils import run_bass_kernel_spmd

    if "nc" not in _CACHE:
        nc = build_nc()
        nc.finalize()
        _CACHE["nc"] = nc
    nc = _CACHE["nc"]

    in_maps = prep_inputs(x, gn_w, gn_b, w_qkv, b_qkv, w_proj, b_proj)
    res = run_bass_kernel_spmd(nc, in_maps, core_ids=list(range(NCORES)))
    out = np.stack([res.results[b]["out"] for b in range(B)], axis=0)
    return out.reshape(B, C, HS, WS).astype(np.float32)


# revision 9
# speedup vs baseline: 1.0936x; 1.0936x over previous
"""AttentionBlock kernel for 8 TRN2 NeuronCores.

Data-parallel over batch: core b computes the full attention block for
batch element b (B=8, one per core). No collectives.

Per-core math (C=512, L=1024, 32 groups, 8 heads, ch=64):
  xn   = GroupNorm(x)                                  [C, L]
  q,k  = W_q xn + b_q, W_k xn + b_k (head-major rows)  [512, L] each
  vT   = xn^T W_v^T + b_v (computed directly as [s,c]) [L, 512]
  WT_h[s,t] = sum_c k_h[c,s] q_h[c,t]                  (per head)
  ET   = exp(WT/8)  (ACT, fused row-sum accum -> N[s])
  a_h[c,t] = sum_s (vT_h[s,c]/N_h[s]) ET_h[s,t]        (softmax over t)
  out  = W_p a + b_p + x

Schedule: attention is processed per head-pair (2j, 2j+1) x s-chunk,
with the AV matmuls delayed one chunk behind the QK/exp front and the
next pair's q/k projection matmuls interleaved one per chunk, so the
PE stays dense (HAM stays warm) while the ACT engine (exp+rowsum, the
per-core bottleneck) is never starved.
"""

import os
import sys

for _p in ("/opt/trn_rl_repo",):
    if os.path.isdir(_p) and _p not in sys.path:
        sys.path.insert(0, _p)

import numpy as np
import ml_dtypes

B, C, HS, WS = 8, 512, 32, 32
L = HS * WS           # 1024
NG = 32               # groups
GSZ = C // NG         # 16 channels per group
NH = 8                # heads
CPH = C // NH         # 64 channels per head
EPS = 1e-5
NCORES = 8

_CACHE = {}


def _head_perm():
    """perm[new] = old row of the 1536-row qkv weight: q all heads, then k, then v."""
    q, k, v = [], [], []
    for h in range(NH):
        base = h * 3 * CPH
        q.extend(range(base, base + CPH))
        k.extend(range(base + CPH, base + 2 * CPH))
        v.extend(range(base + 2 * CPH, base + 3 * CPH))
    return np.array(q + k + v, dtype=np.int64)


def _const_mats():
    # emat[j, p, g] = 1 if group(128j + p) == g   (group stats gather)
    emat = np.zeros((4, 128, NG), dtype=np.float32)
    for j in range(4):
        for p in range(128):
            emat[j, p, (128 * j + p) // GSZ] = 1.0
    # bmat[g, p]: out[p, j] = sum_g bmat[g,p] * R[g,j] -> picks g = 8j + p//16
    bmat = np.zeros((NG, 128), dtype=np.float32)
    for g in range(NG):
        for p in range(128):
            if g % 8 == p // GSZ:
                bmat[g, p] = 1.0
    # m8[g, j] = 1 if g//8 == j  (chunk mask)
    m8 = np.zeros((NG, 4), dtype=np.float32)
    for g in range(NG):
        m8[g, g // 8] = 1.0
    return emat, bmat, m8


def build_nc():
    import concourse.bacc as bacc
    import concourse.tile as tile
    from concourse import mybir

    f32 = mybir.dt.float32
    bf16 = mybir.dt.bfloat16
    AF = mybir.ActivationFunctionType
    OP = mybir.AluOpType

    nc = bacc.Bacc("TRN2", target_bir_lowering=False, debug=False,
                   num_devices=NCORES)

    x_d = nc.declare_dram_parameter("x", [C, L], f32, isOutput=False)
    wqkvT_d = nc.declare_dram_parameter("wqkvT", [C, 3 * C], bf16, isOutput=False)
    wprojT_d = nc.declare_dram_parameter("wprojT", [C, C], bf16, isOutput=False)
    bqk_d = nc.declare_dram_parameter("bqk", [8, 128], f32, isOutput=False)
    bv_d = nc.declare_dram_parameter("bv", [C], f32, isOutput=False)
    gnw_d = nc.declare_dram_parameter("gnw", [4, 128], f32, isOutput=False)
    gnb_d = nc.declare_dram_parameter("gnb", [4, 128], f32, isOutput=False)
    bproj_d = nc.declare_dram_parameter("bproj", [4, 128], f32, isOutput=False)
    emat_d = nc.declare_dram_parameter("emat", [4, 128, NG], f32, isOutput=False)
    bmat_d = nc.declare_dram_parameter("bmat", [NG, 128], f32, isOutput=False)
    m8_d = nc.declare_dram_parameter("m8", [NG, 4], f32, isOutput=False)
    out_d = nc.declare_dram_parameter("out", [C, L], f32, isOutput=True)

    import concourse.bass as bass
    from contextlib import ExitStack

    with tile.TileContext(nc) as tc, ExitStack() as ctx:
        tc.race_detector_enabled = False
        P = ctx.enter_context(tc.tile_pool(name="persist", bufs=1))
        work = ctx.enter_context(tc.tile_pool(name="work", bufs=4))
        etp = ctx.enter_context(tc.tile_pool(name="et", bufs=8))
        ostp = ctx.enter_context(tc.tile_pool(name="ost", bufs=4))
        psA = ctx.enter_context(tc.tile_pool(name="psA", bufs=2, space="PSUM"))
        psB = ctx.enter_context(tc.tile_pool(name="psB", bufs=2, space="PSUM"))

        # ---- persistent tiles
        x_sb = P.tile([128, 4, L], f32, tag="x", name="x")
        xn_sb = P.tile([128, 4, L], bf16, tag="xn", name="xn")
        wq_sb = [P.tile([128, 3 * C], bf16, tag=f"wqkvT{i}", name=f"wqkvT{i}")
                 for i in range(4)]
        wp_sb = [P.tile([128, C], bf16, tag=f"wprojT{i}", name=f"wprojT{i}")
                 for i in range(4)]
        qk_sb = [P.tile([128, L], bf16, tag=f"qk{j}", name=f"qk{j}") for j in range(8)]
        vt_sb = [P.tile([128, C], bf16, tag=f"vt{s}", name=f"vt{s}") for s in range(8)]
        a_sb = [P.tile([128, L], bf16, tag=f"a{j}", name=f"a{j}") for j in range(4)]
        bqk_sb = P.tile([128, 8], f32, tag="bqk", name="bqk")
        bv_sb = P.tile([128, C], f32, tag="bv", name="bv")
        gnw_sb = P.tile([128, 4], f32, tag="gnw", name="gnw")
        gnb_sb = P.tile([128, 4], f32, tag="gnb", name="gnb")
        bproj_sb = P.tile([128, 4], f32, tag="bproj", name="bproj")
        emat_sb = P.tile([128, 4, NG], f32, tag="emat", name="emat")
        bmat_sb = P.tile([NG, 128], f32, tag="bmat", name="bmat")
        m8_sb = P.tile([NG, 4], f32, tag="m8", name="m8")
        zeros_sb = P.tile([128, 1], f32, tag="zeros", name="zeros")
        eps_sb = P.tile([NG, 1], f32, tag="eps", name="eps")
        N_sb = [P.tile([128, 16], f32, tag=f"N{j}", name=f"N{j}") for j in range(4)]
        rN_sb = [P.tile([128, 16], f32, tag=f"rN{j}", name=f"rN{j}")
                 for j in range(4)]
        acc_sb = [P.tile([128, L], f32, tag=f"acc{o}", name=f"acc{o}")
                  for o in range(4)]

        # ---- input DMA (x first: groupnorm is on the critical path)
        for j in range(4):
            for sub in range(2):
                nc.sync.dma_start(
                    out=x_sb[:, j, 512 * sub:512 * (sub + 1)],
                    in_=x_d[128 * j:128 * (j + 1), 512 * sub:512 * (sub + 1)])
        for i in range(4):
            nc.sync.dma_start(out=wq_sb[i], in_=wqkvT_d[128 * i:128 * (i + 1), :])
        nc.sync.dma_start(out=bqk_sb, in_=bqk_d.ap().rearrange("a p -> p a"))
        bv_ap = bass.AP(tensor=bv_d, offset=0, ap=[[0, 128], [1, C]])
        nc.sync.dma_start(out=bv_sb, in_=bv_ap)
        nc.sync.dma_start(out=gnw_sb, in_=gnw_d.ap().rearrange("a p -> p a"))
        nc.sync.dma_start(out=gnb_sb, in_=gnb_d.ap().rearrange("a p -> p a"))
        nc.sync.dma_start(out=bproj_sb, in_=bproj_d.ap().rearrange("a p -> p a"))
        nc.sync.dma_start(out=emat_sb, in_=emat_d.ap().rearrange("a p g -> p a g"))
        nc.sync.dma_start(out=bmat_sb, in_=bmat_d[:, :])
        nc.sync.dma_start(out=m8_sb, in_=m8_d[:, :])
        for i in range(4):
            nc.sync.dma_start(out=wp_sb[i], in_=wprojT_d[128 * i:128 * (i + 1), :])
        nc.vector.memset(zeros_sb, 0.0)
        nc.vector.memset(eps_sb, EPS)
        warm = work.tile([NG, 1], f32, tag="warm", name="warm")
        nc.scalar.activation(out=warm, in_=eps_sb, func=AF.Ln, bias=eps_sb,
                             scale=1.0)

        # ---- phase 1: GroupNorm
        gstats = psA.tile([NG, 2], f32, tag="psA", name="psA_gs")
        for j in range(4):
            st = work.tile([128, 2, 6], f32, tag="bnst", name="bnst")
            for sub in range(2):
                nc.vector.bn_stats(out=st[:, sub, :],
                                   in_=x_sb[:, j, 512 * sub:512 * (sub + 1)])
            mv = work.tile([128, 2], f32, tag="bnmv", name="bnmv")
            nc.vector.bn_aggr(out=mv, in_=st)
            mm2 = work.tile([128, 2], f32, tag="mm2", name="mm2")  # [mean, mean^2+var]
            nc.vector.tensor_copy(out=mm2[:, 0:1], in_=mv[:, 0:1])
            nc.vector.tensor_mul(out=mm2[:, 1:2], in0=mv[:, 0:1], in1=mv[:, 0:1])
            nc.vector.tensor_add(out=mm2[:, 1:2], in0=mm2[:, 1:2], in1=mv[:, 1:2])
            nc.tensor.matmul(gstats, lhsT=emat_sb[:, j, :], rhs=mm2,
                             start=(j == 0), stop=(j == 3))
        gs = work.tile([NG, 2], f32, tag="gs", name="gs")
        nc.vector.tensor_scalar_mul(out=gs, in0=gstats, scalar1=1.0 / GSZ)
        gvar = work.tile([NG, 1], f32, tag="gvar", name="gvar")
        nc.vector.tensor_mul(out=gvar, in0=gs[:, 0:1], in1=gs[:, 0:1])
        nc.vector.tensor_sub(out=gvar, in0=gs[:, 1:2], in1=gvar)
        lnv = work.tile([NG, 1], f32, tag="lnv", name="lnv")
        nc.scalar.activation(out=lnv, in_=gvar, func=AF.Ln, bias=eps_sb, scale=1.0)
        rstd = work.tile([NG, 1], f32, tag="rstd", name="rstd")
        nc.scalar.activation(out=rstd, in_=lnv, func=AF.Exp, bias=zeros_sb[:NG],
                             scale=-0.5)
        R = work.tile([NG, 8], f32, tag="R", name="R")
        nc.vector.tensor_scalar_mul(out=R[:, 0:4], in0=m8_sb, scalar1=gs[:, 0:1])
        nc.vector.tensor_scalar_mul(out=R[:, 4:8], in0=m8_sb, scalar1=rstd)
        pc = psA.tile([128, 8], f32, tag="psA", name="psA_pc")
        nc.tensor.matmul(pc, lhsT=bmat_sb, rhs=R, start=True, stop=True)
        scale = work.tile([128, 4], f32, tag="scale", name="scale")
        shift = work.tile([128, 4], f32, tag="shift", name="shift")
        nc.vector.tensor_mul(out=scale, in0=gnw_sb, in1=pc[:, 4:8])
        nc.vector.tensor_mul(out=shift, in0=pc[:, 0:4], in1=scale)
        nc.vector.tensor_sub(out=shift, in0=gnb_sb, in1=shift)
        for j in range(4):
            nc.vector.tensor_scalar(out=xn_sb[:, j, :], in0=x_sb[:, j, :],
                                    scalar1=scale[:, j:j + 1],
                                    scalar2=shift[:, j:j + 1],
                                    op0=OP.mult, op1=OP.add)

        # helpers -----------------------------------------------------
        pools = [psA, psB]

        def vt_unit(s, pool):
            ps = pool.tile([128, C], f32, tag=pool.name, name="ps_vt")
            for i in range(4):
                nc.tensor.matmul(ps,
                                 lhsT=xn_sb[:, i, 128 * s:128 * (s + 1)],
                                 rhs=wq_sb[i][:, 2 * C:3 * C],
                                 start=(i == 0), stop=(i == 3))
            nc.vector.tensor_add(out=vt_sb[s], in0=ps, in1=bv_sb)

        def qk_half_unit(jj, n, pool):
            ps = pool.tile([128, 512], f32, tag=pool.name, name="ps_qkh")
            for i in range(4):
                nc.tensor.matmul(ps,
                                 lhsT=wq_sb[i][:, 128 * jj:128 * (jj + 1)],
                                 rhs=xn_sb[:, i, 512 * n:512 * (n + 1)],
                                 start=(i == 0), stop=(i == 3))
            nc.vector.tensor_scalar_add(out=qk_sb[jj][:, 512 * n:512 * (n + 1)],
                                        in0=ps, scalar1=bqk_sb[:, jj:jj + 1])

        # ---- phase 2 lead-in: q0/k0 and the first two vT chunks
        for u, (jj, n) in enumerate(((0, 0), (0, 1), (4, 0), (4, 1))):
            qk_half_unit(jj, n, pools[u % 2])
        vt_unit(0, psA)
        vt_unit(1, psB)
        for j in range(4):
            # residual gets b_proj folded in: x <- x + b_proj (off critical path)
            nc.vector.tensor_scalar_add(out=x_sb[:, j, :], in0=x_sb[:, j, :],
                                        scalar1=bproj_sb[:, j:j + 1])

        # ---- phase 3: attention; per pair (heads 2j / 2j+1), AV lags 2 chunks.
        # Remaining vT chunks and the next pair's q/k projections are
        # interleaved as transient psum units to keep the PE dense.
        et_tiles = {}
        vtn_tiles = {}

        def front(j, c):
            for hh, po in ((0, 0), (1, 64)):
                h = 2 * j + hh
                qp = psB.tile([128, L], f32, tag="psB", name="psB_qk")
                for n in range(2):
                    nc.tensor.matmul(qp[:, 512 * n:512 * (n + 1)],
                                     lhsT=qk_sb[4 + j][po:po + 64,
                                                       128 * c:128 * (c + 1)],
                                     rhs=qk_sb[j][po:po + 64,
                                                  512 * n:512 * (n + 1)],
                                     start=True, stop=True)
                et = etp.tile([128, L], bf16, tag="et", name="et")
                nc.scalar.activation(out=et, in_=qp, func=AF.Exp, bias=zeros_sb,
                                     scale=0.125,
                                     accum_out=N_sb[j][:, 2 * c + hh:2 * c + hh + 1])
                et_tiles[(h, c)] = et
            nc.vector.reciprocal(out=rN_sb[j][:, 2 * c:2 * c + 2],
                                 in_=N_sb[j][:, 2 * c:2 * c + 2])
            for hh in (0, 1):
                h = 2 * j + hh
                vtn = work.tile([128, 64], bf16, tag="vtn", name="vtn", bufs=8)
                nc.vector.tensor_scalar_mul(
                    out=vtn, in0=vt_sb[c][:, CPH * h:CPH * (h + 1)],
                    scalar1=rN_sb[j][:, 2 * c + hh:2 * c + hh + 1])
                vtn_tiles[(h, c)] = vtn

        def back(j, c, av):
            for h, po in ((2 * j, 0), (2 * j + 1, 64)):
                vtn = vtn_tiles.pop((h, c))
                et = et_tiles.pop((h, c))
                for n in range(2):
                    nc.tensor.matmul(av[po:po + 64, 512 * n:512 * (n + 1)],
                                     lhsT=vtn, rhs=et[:, 512 * n:512 * (n + 1)],
                                     start=(c == 0), stop=(c == 7),
                                     skip_group_check=True)

        def proj_unit(i, o):
            # partial proj contribution of a_sb[i] to output row-block o
            ps = psA.tile([128, L], f32, tag="psA", name="ps_proj")
            for n in range(2):
                nc.tensor.matmul(ps[:, 512 * n:512 * (n + 1)],
                                 lhsT=wp_sb[i][:, 128 * o:128 * (o + 1)],
                                 rhs=a_sb[i][:, 512 * n:512 * (n + 1)],
                                 start=True, stop=True)
            if i == 0:
                # seed with residual (x already has b_proj folded in)
                nc.vector.tensor_add(out=acc_sb[o], in0=ps, in1=x_sb[:, o, :])
            else:
                nc.vector.tensor_add(out=acc_sb[o], in0=ps, in1=acc_sb[o])

        # insert schedule: pair -> chunk -> list of units
        ins_sched = {
            0: {0: [("vt", 2), ("qk", 1, 0)], 1: [("vt", 3), ("qk", 1, 1)],
                2: [("vt", 4), ("qk", 5, 0)], 3: [("vt", 5), ("qk", 5, 1)],
                4: [("vt", 6)], 5: [("vt", 7)]},
            1: {0: [("qk", 2, 0)], 1: [("qk", 2, 1)],
                2: [("qk", 6, 0)], 3: [("qk", 6, 1)],
                4: [("pr", 0, 0)], 5: [("pr", 0, 1)],
                6: [("pr", 0, 2)], 7: [("pr", 0, 3)]},
            2: {0: [("qk", 3, 0)], 1: [("qk", 3, 1)],
                2: [("qk", 7, 0)], 3: [("qk", 7, 1)],
                4: [("pr", 1, 0)], 5: [("pr", 1, 1)],
                6: [("pr", 1, 2)], 7: [("pr", 1, 3)]},
            3: {0: [("pr", 2, 0)], 1: [("pr", 2, 1)],
                2: [("pr", 2, 2)], 3: [("pr", 2, 3)]},
        }

        DELAY = 2
        for j in range(4):
            av = psA.tile([128, L], f32, tag="psA", name="psA_av")
            for c in range(8):
                front(j, c)
                for unit in ins_sched[j].get(c, ()):
                    if unit[0] == "vt":
                        vt_unit(unit[1], psA)
                    elif unit[0] == "pr":
                        proj_unit(unit[1], unit[2])
                    else:
                        qk_half_unit(unit[1], unit[2], psA)
                if c >= DELAY:
                    back(j, c - DELAY, av)
            for c in range(8 - DELAY, 8):
                back(j, c, av)
            nc.vector.tensor_copy(out=a_sb[j], in_=av)

        # ---- phase 4: last proj partial + store
        for o in range(4):
            ps = psA.tile([128, L], f32, tag="psA", name="psA_pr")
            for n in range(2):
                nc.tensor.matmul(ps[:, 512 * n:512 * (n + 1)],
                                 lhsT=wp_sb[3][:, 128 * o:128 * (o + 1)],
                                 rhs=a_sb[3][:, 512 * n:512 * (n + 1)],
                                 start=True, stop=True)
            ot = ostp.tile([128, L], f32, tag="ost", name="ost")
            nc.vector.tensor_add(out=ot, in0=ps, in1=acc_sb[o])
            nc.sync.dma_start(out=out_d[128 * o:128 * (o + 1), :], in_=ot)

    return nc


def prep_inputs(x, gn_w, gn_b, w_qkv, b_qkv, w_proj, b_proj):
    """Host-side prep: permute/transpose/cast; returns per-core in_maps."""
    x = np.asarray(x, dtype=np.float32)
    gn_w = np.asarray(gn_w, dtype=np.float32)
    gn_b = np.asarray(gn_b, dtype=np.float32)
    w_qkv = np.asarray(w_qkv, dtype=np.float32)
    b_qkv = np.asarray(b_qkv, dtype=np.float32)
    w_proj = np.asarray(w_proj, dtype=np.float32)
    b_proj = np.asarray(b_proj, dtype=np.float32)

    perm = _head_perm()
    wqkvT = np.ascontiguousarray(w_qkv[perm].T).astype(ml_dtypes.bfloat16)
    b_perm = b_qkv[perm]
    wprojT = np.ascontiguousarray(w_proj.T).astype(ml_dtypes.bfloat16)
    emat, bmat, m8 = _const_mats()

    shared = {
        "wqkvT": wqkvT,
        "wprojT": wprojT,
        "bqk": np.ascontiguousarray(b_perm[:1024].reshape(8, 128)),
        "bv": np.ascontiguousarray(b_perm[1024:]),
        "gnw": np.ascontiguousarray(gn_w.reshape(4, 128)),
        "gnb": np.ascontiguousarray(gn_b.reshape(4, 128)),
        "bproj": np.ascontiguousarray(b_proj.reshape(4, 128)),
        "emat": emat, "bmat": bmat, "m8": m8,
    }
    xf = x.reshape(B, C, L)
    in_maps = [dict(shared, x=np.ascontiguousarray(xf[b])) for b in range(B)]
    return in_maps


def kernel(x, gn_w, gn_b, w_qkv, b_qkv, w_proj, b_proj):
    from concourse.bass_utils import run_bass_kernel_spmd

    if "nc" not in _CACHE:
        nc = build_nc()
        nc.finalize()
        _CACHE["nc"] = nc
    nc = _CACHE["nc"]

    in_maps = prep_inputs(x, gn_w, gn_b, w_qkv, b_qkv, w_proj, b_proj)
    res = run_bass_kernel_spmd(nc, in_maps, core_ids=list(range(NCORES)))
    out = np.stack([res.results[b]["out"] for b in range(B)], axis=0)
    return out.reshape(B, C, HS, WS).astype(np.float32)


# revision 10
# speedup vs baseline: 1.2857x; 1.1756x over previous
"""AttentionBlock kernel for 8 TRN2 NeuronCores.

Data-parallel over batch: core b computes the full attention block for
batch element b (B=8, one per core). No collectives.

Per-core math (C=512, L=1024, 32 groups, 8 heads, ch=64):
  xn   = GroupNorm(x)                                  [C, L]
  q,k  = W_q xn + b_q, W_k xn + b_k (head-major rows)  [512, L] each
  vT   = xn^T W_v^T + b_v (computed directly as [s,c]) [L, 512]
  WT_h[s,t] = sum_c k_h[c,s] q_h[c,t]                  (per head)
  ET   = exp(WT/8)  (ACT, fused row-sum accum -> N[s])
  a_h[c,t] = sum_s (vT_h[s,c]/N_h[s]) ET_h[s,t]        (softmax over t)
  out  = W_p a + b_p + x

Schedule: attention is processed per head-pair (2j, 2j+1) x s-chunk,
with the AV matmuls delayed one chunk behind the QK/exp front and the
next pair's q/k projection matmuls interleaved one per chunk, so the
PE stays dense (HAM stays warm) while the ACT engine (exp+rowsum, the
per-core bottleneck) is never starved.
"""

import os
import sys

for _p in ("/opt/trn_rl_repo",):
    if os.path.isdir(_p) and _p not in sys.path:
        sys.path.insert(0, _p)

import numpy as np
import ml_dtypes

B, C, HS, WS = 8, 512, 32, 32
L = HS * WS           # 1024
NG = 32               # groups
GSZ = C // NG         # 16 channels per group
NH = 8                # heads
CPH = C // NH         # 64 channels per head
EPS = 1e-5
NCORES = 8

_CACHE = {}


def _head_perm():
    """perm[new] = old row of the 1536-row qkv weight: q all heads, then k, then v."""
    q, k, v = [], [], []
    for h in range(NH):
        base = h * 3 * CPH
        q.extend(range(base, base + CPH))
        k.extend(range(base + CPH, base + 2 * CPH))
        v.extend(range(base + 2 * CPH, base + 3 * CPH))
    return np.array(q + k + v, dtype=np.int64)


def _const_mats():
    # emat[j, p, g] = 1 if group(128j + p) == g   (group stats gather)
    emat = np.zeros((4, 128, NG), dtype=np.float32)
    for j in range(4):
        for p in range(128):
            emat[j, p, (128 * j + p) // GSZ] = 1.0
    # bmat[g, p]: out[p, j] = sum_g bmat[g,p] * R[g,j] -> picks g = 8j + p//16
    bmat = np.zeros((NG, 128), dtype=np.float32)
    for g in range(NG):
        for p in range(128):
            if g % 8 == p // GSZ:
                bmat[g, p] = 1.0
    # m8[g, j] = 1 if g//8 == j  (chunk mask)
    m8 = np.zeros((NG, 4), dtype=np.float32)
    for g in range(NG):
        m8[g, g // 8] = 1.0
    return emat, bmat, m8


def build_nc():
    import concourse.bacc as bacc
    import concourse.tile as tile
    from concourse import mybir

    f32 = mybir.dt.float32
    bf16 = mybir.dt.bfloat16
    AF = mybir.ActivationFunctionType
    OP = mybir.AluOpType

    nc = bacc.Bacc("TRN2", target_bir_lowering=False, debug=False,
                   num_devices=NCORES)

    x_d = nc.declare_dram_parameter("x", [C, L], f32, isOutput=False)
    wqkvT_d = nc.declare_dram_parameter("wqkvT", [C, 3 * C], bf16, isOutput=False)
    wprojT_d = nc.declare_dram_parameter("wprojT", [C, C], bf16, isOutput=False)
    bqk_d = nc.declare_dram_parameter("bqk", [8, 128], f32, isOutput=False)
    bv_d = nc.declare_dram_parameter("bv", [C], f32, isOutput=False)
    gnw_d = nc.declare_dram_parameter("gnw", [4, 128], f32, isOutput=False)
    gnb_d = nc.declare_dram_parameter("gnb", [4, 128], f32, isOutput=False)
    bproj_d = nc.declare_dram_parameter("bproj", [4, 128], f32, isOutput=False)
    emat_d = nc.declare_dram_parameter("emat", [4, 128, NG], f32, isOutput=False)
    bmat_d = nc.declare_dram_parameter("bmat", [NG, 128], f32, isOutput=False)
    m8_d = nc.declare_dram_parameter("m8", [NG, 4], f32, isOutput=False)
    out_d = nc.declare_dram_parameter("out", [C, L], f32, isOutput=True)

    import concourse.bass as bass
    from contextlib import ExitStack

    with tile.TileContext(nc) as tc, ExitStack() as ctx:
        tc.race_detector_enabled = False
        P = ctx.enter_context(tc.tile_pool(name="persist", bufs=1))
        work = ctx.enter_context(tc.tile_pool(name="work", bufs=4))
        etp = ctx.enter_context(tc.tile_pool(name="et", bufs=8))
        ostp = ctx.enter_context(tc.tile_pool(name="ost", bufs=4))
        psA = ctx.enter_context(tc.tile_pool(name="psA", bufs=2, space="PSUM"))
        psB = ctx.enter_context(tc.tile_pool(name="psB", bufs=2, space="PSUM"))

        # ---- persistent tiles
        x_sb = P.tile([128, 4, L], f32, tag="x", name="x")
        xn_sb = P.tile([128, 4, L], bf16, tag="xn", name="xn")
        wq_sb = [P.tile([128, 3 * C], bf16, tag=f"wqkvT{i}", name=f"wqkvT{i}")
                 for i in range(4)]
        wp_sb = [P.tile([128, C], bf16, tag=f"wprojT{i}", name=f"wprojT{i}")
                 for i in range(4)]
        qk_sb = [P.tile([128, L], bf16, tag=f"qk{j}", name=f"qk{j}") for j in range(8)]
        vt_sb = [P.tile([128, C], bf16, tag=f"vt{s}", name=f"vt{s}") for s in range(8)]
        a_sb = [P.tile([128, L], bf16, tag=f"a{j}", name=f"a{j}") for j in range(4)]
        bqk_sb = P.tile([128, 8], f32, tag="bqk", name="bqk")
        bv_sb = P.tile([128, C], f32, tag="bv", name="bv")
        gnw_sb = P.tile([128, 4], f32, tag="gnw", name="gnw")
        gnb_sb = P.tile([128, 4], f32, tag="gnb", name="gnb")
        bproj_sb = P.tile([128, 4], f32, tag="bproj", name="bproj")
        emat_sb = P.tile([128, 4, NG], f32, tag="emat", name="emat")
        bmat_sb = P.tile([NG, 128], f32, tag="bmat", name="bmat")
        m8_sb = P.tile([NG, 4], f32, tag="m8", name="m8")
        zeros_sb = P.tile([128, 1], f32, tag="zeros", name="zeros")
        eps_sb = P.tile([NG, 1], f32, tag="eps", name="eps")
        N_sb = [P.tile([128, 8], f32, tag=f"N{h}", name=f"N{h}") for h in range(NH)]
        rN_sb = [P.tile([128, 8], f32, tag=f"rN{h}", name=f"rN{h}")
                 for h in range(NH)]

        # ---- input DMA (x first: groupnorm is on the critical path)
        for j in range(4):
            for sub in range(2):
                nc.sync.dma_start(
                    out=x_sb[:, j, 512 * sub:512 * (sub + 1)],
                    in_=x_d[128 * j:128 * (j + 1), 512 * sub:512 * (sub + 1)])
        for i in range(4):
            nc.sync.dma_start(out=wq_sb[i], in_=wqkvT_d[128 * i:128 * (i + 1), :])
        nc.sync.dma_start(out=bqk_sb, in_=bqk_d.ap().rearrange("a p -> p a"))
        bv_ap = bass.AP(tensor=bv_d, offset=0, ap=[[0, 128], [1, C]])
        nc.sync.dma_start(out=bv_sb, in_=bv_ap)
        nc.sync.dma_start(out=gnw_sb, in_=gnw_d.ap().rearrange("a p -> p a"))
        nc.sync.dma_start(out=gnb_sb, in_=gnb_d.ap().rearrange("a p -> p a"))
        nc.sync.dma_start(out=bproj_sb, in_=bproj_d.ap().rearrange("a p -> p a"))
        nc.sync.dma_start(out=emat_sb, in_=emat_d.ap().rearrange("a p g -> p a g"))
        nc.sync.dma_start(out=bmat_sb, in_=bmat_d[:, :])
        nc.sync.dma_start(out=m8_sb, in_=m8_d[:, :])
        for i in range(4):
            nc.sync.dma_start(out=wp_sb[i], in_=wprojT_d[128 * i:128 * (i + 1), :])
        nc.vector.memset(zeros_sb, 0.0)
        nc.vector.memset(eps_sb, EPS)
        warm = work.tile([NG, 1], f32, tag="warm", name="warm")
        nc.scalar.activation(out=warm, in_=eps_sb, func=AF.Ln, bias=eps_sb,
                             scale=1.0)

        # ---- phase 1: GroupNorm
        gstats = psA.tile([NG, 2], f32, tag="psA", name="psA_gs")
        for j in range(4):
            st = work.tile([128, 2, 6], f32, tag="bnst", name="bnst")
            for sub in range(2):
                nc.vector.bn_stats(out=st[:, sub, :],
                                   in_=x_sb[:, j, 512 * sub:512 * (sub + 1)])
            mv = work.tile([128, 2], f32, tag="bnmv", name="bnmv")
            nc.vector.bn_aggr(out=mv, in_=st)
            mm2 = work.tile([128, 2], f32, tag="mm2", name="mm2")  # [mean, mean^2+var]
            nc.vector.tensor_copy(out=mm2[:, 0:1], in_=mv[:, 0:1])
            nc.vector.tensor_mul(out=mm2[:, 1:2], in0=mv[:, 0:1], in1=mv[:, 0:1])
            nc.vector.tensor_add(out=mm2[:, 1:2], in0=mm2[:, 1:2], in1=mv[:, 1:2])
            nc.tensor.matmul(gstats, lhsT=emat_sb[:, j, :], rhs=mm2,
                             start=(j == 0), stop=(j == 3))
        gs = work.tile([NG, 2], f32, tag="gs", name="gs")
        nc.vector.tensor_scalar_mul(out=gs, in0=gstats, scalar1=1.0 / GSZ)
        gvar = work.tile([NG, 1], f32, tag="gvar", name="gvar")
        nc.vector.tensor_mul(out=gvar, in0=gs[:, 0:1], in1=gs[:, 0:1])
        nc.vector.tensor_sub(out=gvar, in0=gs[:, 1:2], in1=gvar)
        lnv = work.tile([NG, 1], f32, tag="lnv", name="lnv")
        nc.scalar.activation(out=lnv, in_=gvar, func=AF.Ln, bias=eps_sb, scale=1.0)
        rstd = work.tile([NG, 1], f32, tag="rstd", name="rstd")
        nc.scalar.activation(out=rstd, in_=lnv, func=AF.Exp, bias=zeros_sb[:NG],
                             scale=-0.5)
        R = work.tile([NG, 8], f32, tag="R", name="R")
        nc.vector.tensor_scalar_mul(out=R[:, 0:4], in0=m8_sb, scalar1=gs[:, 0:1])
        nc.vector.tensor_scalar_mul(out=R[:, 4:8], in0=m8_sb, scalar1=rstd)
        pc = psA.tile([128, 8], f32, tag="psA", name="psA_pc")
        nc.tensor.matmul(pc, lhsT=bmat_sb, rhs=R, start=True, stop=True)
        scale = work.tile([128, 4], f32, tag="scale", name="scale")
        shift = work.tile([128, 4], f32, tag="shift", name="shift")
        nc.vector.tensor_mul(out=scale, in0=gnw_sb, in1=pc[:, 4:8])
        nc.vector.tensor_mul(out=shift, in0=pc[:, 0:4], in1=scale)
        nc.vector.tensor_sub(out=shift, in0=gnb_sb, in1=shift)
        for j in range(4):
            nc.vector.tensor_scalar(out=xn_sb[:, j, :], in0=x_sb[:, j, :],
                                    scalar1=scale[:, j:j + 1],
                                    scalar2=shift[:, j:j + 1],
                                    op0=OP.mult, op1=OP.add)

        # helpers -----------------------------------------------------
        pools = [psA, psB]

        def vt_unit(s, pool):
            ps = pool.tile([128, C], f32, tag=pool.name, name="ps_vt")
            for i in range(4):
                nc.tensor.matmul(ps,
                                 lhsT=xn_sb[:, i, 128 * s:128 * (s + 1)],
                                 rhs=wq_sb[i][:, 2 * C:3 * C],
                                 start=(i == 0), stop=(i == 3))
            nc.vector.tensor_add(out=vt_sb[s], in0=ps, in1=bv_sb)

        def qk_half_unit(jj, n, pool):
            ps = pool.tile([128, 512], f32, tag=pool.name, name="ps_qkh")
            for i in range(4):
                nc.tensor.matmul(ps,
                                 lhsT=wq_sb[i][:, 128 * jj:128 * (jj + 1)],
                                 rhs=xn_sb[:, i, 512 * n:512 * (n + 1)],
                                 start=(i == 0), stop=(i == 3))
            nc.vector.tensor_scalar_add(out=qk_sb[jj][:, 512 * n:512 * (n + 1)],
                                        in0=ps, scalar1=bqk_sb[:, jj:jj + 1])

        # ---- phase 2 lead-in: q0/k0 and the first two vT chunks
        for u, (jj, n) in enumerate(((0, 0), (0, 1), (4, 0), (4, 1))):
            qk_half_unit(jj, n, pools[u % 2])
        vt_unit(0, psA)
        vt_unit(1, psB)
        for j in range(4):
            # residual gets b_proj folded in: x <- x + b_proj (off critical path)
            nc.vector.tensor_scalar_add(out=x_sb[:, j, :], in0=x_sb[:, j, :],
                                        scalar1=bproj_sb[:, j:j + 1])

        # ---- phase 3: attention; per pair (heads 2j / 2j+1), AV lags 2 chunks.
        # Remaining vT chunks and the next pair's q/k projections are
        # interleaved as transient psum units to keep the PE dense.
        et_tiles = {}
        vtn_tiles = {}

        def front(j, c):
            for h, po in ((2 * j, 0), (2 * j + 1, 64)):
                qp = psB.tile([128, L], f32, tag="psB", name="psB_qk")
                for n in range(2):
                    nc.tensor.matmul(qp[:, 512 * n:512 * (n + 1)],
                                     lhsT=qk_sb[4 + j][po:po + 64,
                                                       128 * c:128 * (c + 1)],
                                     rhs=qk_sb[j][po:po + 64,
                                                  512 * n:512 * (n + 1)],
                                     start=True, stop=True)
                et = etp.tile([128, L], bf16, tag="et", name="et")
                nc.scalar.activation(out=et, in_=qp, func=AF.Exp, bias=zeros_sb,
                                     scale=0.125, accum_out=N_sb[h][:, c:c + 1])
                nc.vector.reciprocal(out=rN_sb[h][:, c:c + 1],
                                     in_=N_sb[h][:, c:c + 1])
                vtn = work.tile([128, 64], bf16, tag="vtn", name="vtn", bufs=8)
                nc.vector.tensor_scalar_mul(out=vtn,
                                            in0=vt_sb[c][:, CPH * h:CPH * (h + 1)],
                                            scalar1=rN_sb[h][:, c:c + 1])
                et_tiles[(h, c)] = et
                vtn_tiles[(h, c)] = vtn

        def back(j, c, av):
            for h, po in ((2 * j, 0), (2 * j + 1, 64)):
                vtn = vtn_tiles.pop((h, c))
                et = et_tiles.pop((h, c))
                for n in range(2):
                    nc.tensor.matmul(av[po:po + 64, 512 * n:512 * (n + 1)],
                                     lhsT=vtn, rhs=et[:, 512 * n:512 * (n + 1)],
                                     start=(c == 0), stop=(c == 7),
                                     skip_group_check=True)

        # insert schedule: pair -> chunk -> list of units
        ins_sched = {
            0: {0: [("vt", 2), ("qk", 1, 0)], 1: [("vt", 3), ("qk", 1, 1)],
                2: [("vt", 4), ("qk", 5, 0)], 3: [("vt", 5), ("qk", 5, 1)],
                4: [("vt", 6)], 5: [("vt", 7)]},
            1: {0: [("qk", 2, 0)], 1: [("qk", 2, 1)],
                2: [("qk", 6, 0)], 3: [("qk", 6, 1)]},
            2: {0: [("qk", 3, 0)], 1: [("qk", 3, 1)],
                2: [("qk", 7, 0)], 3: [("qk", 7, 1)]},
            3: {},
        }

        DELAY = 2
        for j in range(4):
            av = psA.tile([128, L], f32, tag="psA", name="psA_av")
            for c in range(8):
                front(j, c)
                for unit in ins_sched[j].get(c, ()):
                    if unit[0] == "vt":
                        vt_unit(unit[1], psA)
                    else:
                        qk_half_unit(unit[1], unit[2], psA)
                if c >= DELAY:
                    back(j, c - DELAY, av)
            for c in range(8 - DELAY, 8):
                back(j, c, av)
            nc.vector.tensor_copy(out=a_sb[j], in_=av)

        # ---- phase 4: proj + residual
        for o in range(4):
            ps = psA.tile([128, L], f32, tag="psA", name="psA_pr")
            for i in range(4):
                for n in range(2):
                    nc.tensor.matmul(ps[:, 512 * n:512 * (n + 1)],
                                     lhsT=wp_sb[i][:, 128 * o:128 * (o + 1)],
                                     rhs=a_sb[i][:, 512 * n:512 * (n + 1)],
                                     start=(i == 0), stop=(i == 3))
            ot = ostp.tile([128, L], f32, tag="ost", name="ost")
            nc.vector.tensor_add(out=ot, in0=ps, in1=x_sb[:, o, :])
            nc.sync.dma_start(out=out_d[128 * o:128 * (o + 1), :], in_=ot)

    return nc


def prep_inputs(x, gn_w, gn_b, w_qkv, b_qkv, w_proj, b_proj):
    """Host-side prep: permute/transpose/cast; returns per-core in_maps."""
    x = np.asarray(x, dtype=np.float32)
    gn_w = np.asarray(gn_w, dtype=np.float32)
    gn_b = np.asarray(gn_b, dtype=np.float32)
    w_qkv = np.asarray(w_qkv, dtype=np.float32)
    b_qkv = np.asarray(b_qkv, dtype=np.float32)
    w_proj = np.asarray(w_proj, dtype=np.float32)
    b_proj = np.asarray(b_proj, dtype=np.float32)

    perm = _head_perm()
    wqkvT = np.ascontiguousarray(w_qkv[perm].T).astype(ml_dtypes.bfloat16)
    b_perm = b_qkv[perm]
    wprojT = np.ascontiguousarray(w_proj.T).astype(ml_dtypes.bfloat16)
    emat, bmat, m8 = _const_mats()

    shared = {
        "wqkvT": wqkvT,
        "wprojT": wprojT,
        "bqk": np.ascontiguousarray(b_perm[:1024].reshape(8, 128)),
        "bv": np.ascontiguousarray(b_perm[1024:]),
        "gnw": np.ascontiguousarray(gn_w.reshape(4, 128)),
        "gnb": np.ascontiguousarray(gn_b.reshape(4, 128)),
        "bproj": np.ascontiguousarray(b_proj.reshape(4, 128)),
        "emat": emat, "bmat": bmat, "m8": m8,
    }
    xf = x.reshape(B, C, L)
    in_maps = [dict(shared, x=np.ascontiguousarray(xf[b])) for b in range(B)]
    return in_maps


def kernel(x, gn_w, gn_b, w_qkv, b_qkv, w_proj, b_proj):
    from concourse.bass_utils import run_bass_kernel_spmd

    if "nc" not in _CACHE:
        nc = build_nc()
        nc.finalize()
        _CACHE["nc"] = nc
    nc = _CACHE["nc"]

    in_maps = prep_inputs(x, gn_w, gn_b, w_qkv, b_qkv, w_proj, b_proj)
    res = run_bass_kernel_spmd(nc, in_maps, core_ids=list(range(NCORES)))
    out = np.stack([res.results[b]["out"] for b in range(B)], axis=0)
    return out.reshape(B, C, HS, WS).astype(np.float32)
